# revision 15
# baseline (speedup 1.0000x reference)
"""BackgroundNoiseLayer kernel for 8 trn2 NeuronCores.

Math: out[0, t, n] = sum_k W[n, k] * rest[t, k], where W [60000, 100] is
scatter-added from COO (v1 block rows 0..49999, lm block rows 50000..59999)
and the output feature axis is the concat of the two blocks.

Strategy (per sharding hint): densify the tiny sparse matrix host-side
(240k nnz -> dense W, ~0.002% of the matmul FLOPs), shard the post-synaptic
feature axis across the 8 cores (7500 features each), and run a dense
[1000,101] @ [101,7500] matmul per core. rest is tiny and replicated. Each
core writes its own contiguous output slice; concat on host is the no-op
gather.

Precision scheme (gate is rel_err < 2e-2): the device emits a per-feature
scaled int8 stream. Host folds 127/s_n into W (s_n = 5 sigma of feature n,
computed exactly from the actual rest moments) and appends a constant-1
column to rest carrying -127*mu_n/s_n, so PSUM holds the centered, scaled
value in [-127,127]. The copy out of PSUM casts f32->int8; host decodes
q*(s/127)+mu in f32. Measured rel err ~9e-3 (numpy sim, round-to-nearest).
int8 halves HBM write bytes vs fp16 - this pays because SDMA engine 15 is
chronically ~10% slower than its 15 peers and its cumulative backlog sets
the post-stream drain time.

Device-side pipeline (from profiling):
- Three stations: PE (bf16 matmuls, N=512 per PSUM bank, HAM pre-warmed by
  8 dummy matmuls issued before any DMA so the real stream runs at 2.4
  GHz from its first instruction), PSUM evacuation (DVE+ACT alternating on
  [128,1024] double-bank PSUM tiles - the saturated station at ~36 us),
  out-DMA on the sync HWDGE ring (~400 GB/s with 2+ DMAs in flight).
- Every block writes out as two half-DMAs from two separate stage tiles
  so the first half's transfer overlaps the second half's copies.
- Last double-chunk is 332 real columns (7500 = 7*1024 + 332): no padded
  compute/copy/DMA on the feature axis.
- sync ring carries only the output stream (plus the first row-block's
  inputs to cut time-to-first-matmul); other input loads ride the gpsimd
  SWDGE ring.
"""

import os

import numpy as np

B, T = 1, 1000
NBKG = 100
NV1, NLM = 50000, 10000
NPOST = NV1 + NLM          # 60000
NCORES = 8
SHARD = NPOST // NCORES    # 7500 real features per core

KP = 112                   # padded contraction dim (100 real + 1 bias col)
ROWS = 1024                # padded time rows (zeros in 1000..1023)
TBLK = 128                 # rows per block = full partition set
NT = ROWS // TBLK          # 8
MMN = 512                  # matmul free dim cap = one fp32 PSUM bank
DCW = 1024                 # double-chunk width (2 PSUM banks)
DCHUNKS = [(i * DCW, DCW) for i in range(7)] + [(7 * DCW, SHARD - 7 * DCW)]
ALPHA = 5.0                # int8 scale: s_n = ALPHA * sigma_n

_compiled = None


def _build_module():
    import concourse.bacc as bacc
    import concourse.mybir as mybir
    import concourse.tile as tile

    f32 = mybir.dt.float32
    i8 = mybir.dt.int8
    bf16 = mybir.dt.bfloat16
    nc = bacc.Bacc("TRN2", target_bir_lowering=False, debug=False)
    restT = nc.dram_tensor("restT", [KP, ROWS], bf16, kind="ExternalInput")
    wT = nc.dram_tensor("wT", [KP, SHARD], bf16, kind="ExternalInput")
    out = nc.dram_tensor("out", [ROWS, SHARD], i8, kind="ExternalOutput")

    with tile.TileContext(nc) as tc:
        with (
            tc.tile_pool(name="inp", bufs=1) as inp,
            tc.tile_pool(name="stage", bufs=3) as stagep,
            tc.tile_pool(name="psum", bufs=4, space="PSUM") as psump,
        ):
            # ~8 dummy matmuls on a zeroed scratch tile FIRST (before any
            # DMA issues occupy the queues): PE activity from ~6.3us warms
            # the HAM clock gate by ~11us, so the real matmul stream runs
            # at 2.4 GHz from its first instruction instead of spending
            # its first ~4us at 1.2 GHz
            scratch = inp.tile([KP, MMN], bf16, tag="warm")
            nc.vector.memset(scratch[:], 0.0)
            for _ in range(8):
                psw = psump.tile([TBLK, DCW], f32, tag="ps")
                nc.tensor.matmul(psw[:, :MMN], scratch[:, :TBLK],
                                 scratch[:], start=True, stop=True)

            # input loads, size-ramped so block 0's warm-rate consumption
            # (chunk j needed at ~11.4 + j*0.64 us) never outruns arrival:
            # small early tiles (w0 alone on sync lands ~8.3us, w1 behind
            # rest0 on scalar ~8.5us), then two big coalesced tiles whose
            # ~400 KB+ transfers beat per-chunk issue serialization (w2-3
            # on gpsimd ready ~12.8, w4-7 on scalar ready ~13.2). rest1
            # loads last on gpsimd; row-block 2 needs it only at ~20us.
            rest0 = inp.tile([KP, 2 * TBLK], bf16, tag="rest0")
            nc.scalar.dma_start(rest0[:], restT[:, :2 * TBLK])
            w0 = inp.tile([KP, DCW], bf16, tag="w0", name="w0")
            nc.sync.dma_start(w0[:], wT[:, :DCW])
            w1 = inp.tile([KP, DCW], bf16, tag="w1", name="w1")
            nc.scalar.dma_start(w1[:], wT[:, DCW:2 * DCW])
            w23 = inp.tile([KP, 2 * DCW], bf16, tag="w23", name="w23")
            nc.gpsimd.dma_start(w23[:], wT[:, 2 * DCW:4 * DCW])
            w47 = inp.tile([KP, SHARD - 4 * DCW], bf16, tag="w47",
                           name="w47")
            nc.scalar.dma_start(w47[:], wT[:, 4 * DCW:])
            rest1 = inp.tile([KP, ROWS - 2 * TBLK], bf16, tag="rest1")
            nc.gpsimd.dma_start(rest1[:], restT[:, 2 * TBLK:])
            # per-chunk view: (tile, column offset within tile)
            wmap = {0: (w0, 0), 1: (w1, 0), 2: (w23, 0), 3: (w23, DCW),
                    4: (w47, 0), 5: (w47, DCW), 6: (w47, 2 * DCW),
                    7: (w47, 3 * DCW)}

            # every block writes out as two half-DMAs from two separate
            # stage tiles. vector takes 3x1024+332 per block, scalar
            # 4x1024.
            HSPLIT = 4 * DCW           # 4096
            vector_chunks = {0, 2, 4, 7}
            for tb in range(NT):
                r0, r1 = tb * TBLK, (tb + 1) * TBLK
                stageA = stagep.tile([TBLK, HSPLIT], i8, tag="stA",
                                     name=f"stA{tb}", bufs=3)
                stageB = stagep.tile([TBLK, SHARD - HSPLIT], i8, tag="stB",
                                     name=f"stB{tb}", bufs=3)
                if tb < 2:
                    lhsT = rest0[:, tb * TBLK:(tb + 1) * TBLK]
                else:
                    lhsT = rest1[:, (tb - 2) * TBLK:(tb - 1) * TBLK]
                for j, (off, w) in enumerate(DCHUNKS):
                    ps = psump.tile([TBLK, DCW], f32, tag="ps")
                    wt, woff = wmap[j]
                    for m in range((w + MMN - 1) // MMN):
                        n0 = m * MMN
                        n1 = min(w, n0 + MMN)
                        nc.tensor.matmul(
                            ps[:, n0:n1],
                            lhsT,
                            wt[:, woff + n0:woff + n1],
                            start=True,
                            stop=True,
                        )
                    copy = (nc.vector.tensor_copy if j in vector_chunks
                            else nc.scalar.copy)
                    if off < HSPLIT:
                        copy(stageA[:, off:off + w], ps[:, :w])
                    else:
                        copy(stageB[:, off - HSPLIT:off - HSPLIT + w],
                             ps[:, :w])
                    if j == 3:
                        nc.sync.dma_start(out[r0:r1, :HSPLIT], stageA[:])
                    elif j == 7:
                        nc.sync.dma_start(out[r0:r1, HSPLIT:], stageB[:])

    nc.compile()
    return nc


def _densify(v1_weights, v1_rows, v1_cols, lm_weights, lm_rows, lm_cols):
    rows = np.concatenate([
        np.asarray(v1_rows).astype(np.int64),
        np.asarray(lm_rows).astype(np.int64) + NV1,
    ])
    cols = np.concatenate([
        np.asarray(v1_cols).astype(np.int64),
        np.asarray(lm_cols).astype(np.int64),
    ])
    w = np.concatenate([
        np.asarray(v1_weights, dtype=np.float32),
        np.asarray(lm_weights, dtype=np.float32),
    ])
    W = np.bincount(rows * NBKG + cols, weights=w, minlength=NPOST * NBKG)
    return W.astype(np.float32).reshape(NPOST, NBKG)


def kernel(rest, v1_weights, v1_rows, v1_cols, lm_weights, lm_rows, lm_cols):
    import ml_dtypes

    from concourse.bass_utils import run_bass_kernel_spmd

    bf16 = ml_dtypes.bfloat16

    global _compiled
    if _compiled is None:
        _compiled = _build_module()

    W = _densify(v1_weights, v1_rows, v1_cols, lm_weights, lm_rows, lm_cols)
    rest32 = np.asarray(rest, np.float32)

    # per-feature affine int8 code: psum = 127*(out - mu)/s, decoded
    # host-side as q*(s/127) + mu. mu and sigma are exact moments of the
    # actual rest sample, so s = ALPHA*sigma covers the deviations.
    lam = rest32.mean(0)                       # [NBKG]
    var = ((rest32 - lam) ** 2).mean(0)        # [NBKG]
    mu = W @ lam                               # [NPOST]
    sig = np.sqrt(np.maximum((W * W) @ var, 1e-12))
    s = ALPHA * sig
    Wq = W * (127.0 / s)[:, None]              # [NPOST, NBKG]
    muq = -127.0 * mu / s                      # [NPOST]

    restT = np.zeros((KP, ROWS), bf16)
    restT[:NBKG, :B * T] = rest32.astype(bf16).T
    restT[NBKG, :B * T] = bf16(1.0)            # bias column

    in_maps = []
    for c in range(NCORES):
        sl = slice(c * SHARD, (c + 1) * SHARD)
        wpad = np.zeros((KP, SHARD), bf16)
        wpad[:NBKG, :] = Wq[sl].T.astype(bf16)
        wpad[NBKG, :] = muq[sl].astype(bf16)
        in_maps.append({"restT": restT, "wT": wpad})

    trace = bool(int(os.environ.get("KERNEL_TRACE", "0")))
    if trace:
        _install_ntff_shim()
    res = run_bass_kernel_spmd(
        _compiled, in_maps, core_ids=list(range(NCORES)), trace=trace
    )
    kernel.last_results = res
    dec = [
        res.results[c]["out"][:B * T, :].astype(np.float32)
        * (s[c * SHARD:(c + 1) * SHARD] / 127.0)[None, :]
        + mu[c * SHARD:(c + 1) * SHARD][None, :]
        for c in range(NCORES)
    ]
    full = np.concatenate(dec, axis=1)
    return full.reshape(B, T, NPOST)


def _install_ntff_shim():
    """The agent image's antenv lacks axon_hooks; register the NTFF profile
    hook by dlopening libaxon_pjrt.so directly (same path trn_boot uses)."""
    import sys
    import types

    if "antenv.axon_hooks" in sys.modules:
        return
    try:
        from trn_agent_boot.trn_boot import _ntff_profile_via_ctypes

        hook = _ntff_profile_via_ctypes("/opt/axon/libaxon_pjrt.so")
    except Exception:
        hook = None
    mod = types.ModuleType("antenv.axon_hooks")
    mod.get_axon_ntff_profile_hook = lambda: hook
    mod.set_axon_ntff_profile_hook = lambda h: None
    sys.modules["antenv.axon_hooks"] = mod
